# revision 1
# baseline (speedup 1.0000x reference)
"""LIF spiking-neuron kernel for Trainium2, data-parallel over 8 NeuronCores.

Reference semantics (T=4, THRESH=1.0, TAU=1.0):
    x: [T*B, N] -> reshape [T, B, N]; mem0 = 0
    per t: mem += x_t; spike_t = (mem >= 1.0); mem *= (1 - spike_t)
    out: spikes reshaped [T*B, N]

Sharding: pure data parallel over B. Core i gets rows i*256:(i+1)*256 of
each timestep block -> shard [T*256, N] = [1024, 4096] f32 in/out per core.

Raw-Bass implementation (the Tile framework's multi-wait instructions don't
pass this container's walrus codegen). Engine split:
  SP (sync, HWDGE)   : all x loads, ring-buffered, prefetch ahead
  ACT (scalar, HWDGE): all spike stores (separate DGE ring so stores
                       waiting on compute never block load prefetch)
  DVE (vector)       : add / is_ge / is_lt / mult
Per chunk instance [128, N]: mem tile persists across the T=4 recurrence;
t=0 loads x0 directly as mem (no memset, no add); reset (is_lt + mult)
skipped at t=3 since mem is dead afterward.
"""

from contextlib import ExitStack

import numpy as np

import concourse.bass as bass
from concourse import mybir
from concourse.bass_utils import run_bass_kernel_spmd

T = 4
B = 2048
N = 4096
N_CORES = 8
BSH = B // N_CORES  # 256 rows per core per timestep
P = 128

F32 = mybir.dt.float32


def build_nc(t_dim=T, bsh=BSH, n=N, bench_iters=None, accum=False):
    """One-core Bass module: x [t*bsh, n] f32 -> out [t*bsh, n] f32.

    bench_iters: if set, repeat the whole (idempotent) program that many
    times with continuing semaphore counts — used only for slope timing.
    accum: if True, fold the mem += x_t adds into SWDGE accumulate-DMA
    loads (gpsimd) targeting the mem tile directly — removes the DVE adds
    and the x ring buffers.
    """
    if accum:
        return _build_nc_accum(t_dim, bsh, n, bench_iters)
    pb = bsh // P  # spatial chunks of [128, n]
    assert bsh % P == 0
    reps = bench_iters or 1
    ng = pb * reps  # chunk instances
    nu = t_dim * ng  # (instance, t) units
    NXB = 3  # x-tile ring
    NSB = 3  # spike-tile ring
    NMEM = 2  # mem/mask rings (one per in-flight chunk)

    nc = bass.Bass()
    x = nc.declare_dram_parameter("x", [t_dim * bsh, n], F32, isOutput=False)
    out = nc.declare_dram_parameter("out", [t_dim * bsh, n], F32, isOutput=True)
    xv = x.rearrange("(t pb p) n -> t pb p n", t=t_dim, pb=pb, p=P)
    ov = out.rearrange("(t pb p) n -> t pb p n", t=t_dim, pb=pb, p=P)

    # --- precompute DVE program order so waits can reference exact counts.
    # v counts DVE instructions (each increments v_sem by 1).
    vidx_ge = {}  # unit u -> v count after its is_ge
    vidx_add = {}  # x-load j -> v count after the add that consumes it
    vidx_last = {}  # instance g -> v count after its final DVE op
    v = 0
    for g in range(ng):
        for t in range(t_dim):
            u = t_dim * g + t
            if t > 0:
                v += 1  # add
                vidx_add[(t_dim - 1) * g + (t - 1)] = v
            v += 1  # is_ge
            vidx_ge[u] = v
            if t < t_dim - 1:
                v += 2  # is_lt, mult
        vidx_last[g] = v

    with ExitStack() as ctx:
        mem = [
            ctx.enter_context(nc.sbuf_tensor(f"mem{i}", [P, n], F32))
            for i in range(NMEM)
        ]
        msk = [
            ctx.enter_context(nc.sbuf_tensor(f"msk{i}", [P, n], F32))
            for i in range(NMEM)
        ]
        xb = [
            ctx.enter_context(nc.sbuf_tensor(f"xb{i}", [P, n], F32))
            for i in range(NXB)
        ]
        sb = [
            ctx.enter_context(nc.sbuf_tensor(f"sb{i}", [P, n], F32))
            for i in range(NSB)
        ]
        # One semaphore per ring slot: concurrent DMA completions interleave
        # their 16 per-engine increments, so a shared cumulative sem cannot
        # identify which DMA finished. Per-slot sems are unambiguous because
        # a slot's next DMA is only issued after its previous reader ran.
        mem_sem = [
            ctx.enter_context(nc.semaphore(f"mem_sem{i}")) for i in range(NMEM)
        ]
        xb_sem = [
            ctx.enter_context(nc.semaphore(f"xb_sem{i}")) for i in range(NXB)
        ]
        sb_sem = [
            ctx.enter_context(nc.semaphore(f"sb_sem{i}")) for i in range(NSB)
        ]
        v_sem = ctx.enter_context(nc.semaphore("v_sem"))
        block = ctx.enter_context(nc.Block())

        @block.sync
        def _(sync):
            for g in range(ng):
                c = g % pb
                if g >= NMEM:  # WAR: mem slot still read by instance g-NMEM
                    sync.wait_ge(v_sem, vidx_last[g - NMEM])
                sync.dma_start(mem[g % NMEM][:], xv[0, c]).then_inc(
                    mem_sem[g % NMEM], 16
                )
                for t in range(1, t_dim):
                    j = (t_dim - 1) * g + (t - 1)
                    if j >= NXB:  # WAR: x slot still read by add j-NXB
                        sync.wait_ge(v_sem, vidx_add[j - NXB])
                    sync.dma_start(xb[j % NXB][:], xv[t, c]).then_inc(
                        xb_sem[j % NXB], 16
                    )

        @block.vector
        def _(vector):
            # DVE is one dependent chain through mem per chunk; wait for all
            # prior DVE ops before each op (same-engine sem waits are
            # already satisfied at issue time, so this costs nothing but
            # guarantees SBUF write visibility across the deep pipeline).
            v = 0

            def dve(ins):
                nonlocal v
                v += 1
                ins.then_inc(v_sem, 1)

            for g in range(ng):
                m = mem[g % NMEM]
                k = msk[g % NMEM]
                for t in range(t_dim):
                    u = t_dim * g + t
                    if t == 0:
                        vector.wait_ge(mem_sem[g % NMEM], 16 * (g // NMEM + 1))
                    else:
                        j = (t_dim - 1) * g + (t - 1)
                        vector.wait_ge(xb_sem[j % NXB], 16 * (j // NXB + 1))
                        vector.wait_ge(v_sem, v)
                        dve(vector.tensor_add(m[:], m[:], xb[j % NXB][:]))
                    if u >= NSB:  # WAR: spike slot still being stored
                        vector.wait_ge(sb_sem[u % NSB], 16 * (u // NSB))
                    vector.wait_ge(v_sem, v)
                    dve(
                        vector.tensor_scalar(
                            sb[u % NSB][:], m[:], 1.0, None, mybir.AluOpType.is_ge
                        )
                    )
                    if t < t_dim - 1:
                        vector.wait_ge(v_sem, v)
                        dve(
                            vector.tensor_scalar(
                                k[:], m[:], 1.0, None, mybir.AluOpType.is_lt
                            )
                        )
                        vector.wait_ge(v_sem, v)
                        dve(vector.tensor_mul(m[:], m[:], k[:]))

        @block.scalar
        def _(scalar):
            for u in range(nu):
                g, t = divmod(u, t_dim)
                c = g % pb
                scalar.wait_ge(v_sem, vidx_ge[u])
                scalar.dma_start(ov[t, c], sb[u % NSB][:]).then_inc(
                    sb_sem[u % NSB], 16
                )
            for i in range(NSB):  # drain: all stores landed before NEFF end
                scalar.wait_ge(sb_sem[i], 16 * ((nu - 1 - i) // NSB + 1))

    return nc


def _build_nc_accum(t_dim, bsh, n, bench_iters):
    """Variant: x_t (t>=1) is added to mem by the DMA engines (CCE add)."""
    pb = bsh // P
    assert bsh % P == 0
    reps = bench_iters or 1
    ng = pb * reps
    nu = t_dim * ng
    NSB = 3
    NMEM = 2

    nc = bass.Bass()
    x = nc.declare_dram_parameter("x", [t_dim * bsh, n], F32, isOutput=False)
    out = nc.declare_dram_parameter("out", [t_dim * bsh, n], F32, isOutput=True)
    xv = x.rearrange("(t pb p) n -> t pb p n", t=t_dim, pb=pb, p=P)
    ov = out.rearrange("(t pb p) n -> t pb p n", t=t_dim, pb=pb, p=P)

    # DVE program: per (g, t): is_ge, then (t < T-1) is_lt + mult.
    vidx_ge = {}
    vidx_mult = {}  # (g, t) -> v count after mult of step t
    vidx_last = {}
    v = 0
    for g in range(ng):
        for t in range(t_dim):
            u = t_dim * g + t
            v += 1  # is_ge
            vidx_ge[u] = v
            if t < t_dim - 1:
                v += 2  # is_lt, mult
                vidx_mult[(g, t)] = v
        vidx_last[g] = v

    with ExitStack() as ctx:
        mem = [
            ctx.enter_context(nc.sbuf_tensor(f"mem{i}", [P, n], F32))
            for i in range(NMEM)
        ]
        msk = [
            ctx.enter_context(nc.sbuf_tensor(f"msk{i}", [P, n], F32))
            for i in range(NMEM)
        ]
        sb = [
            ctx.enter_context(nc.sbuf_tensor(f"sb{i}", [P, n], F32))
            for i in range(NSB)
        ]
        mem_sem = [
            ctx.enter_context(nc.semaphore(f"mem_sem{i}")) for i in range(NMEM)
        ]
        sb_sem = [
            ctx.enter_context(nc.semaphore(f"sb_sem{i}")) for i in range(NSB)
        ]
        v_sem = ctx.enter_context(nc.semaphore("v_sem"))
        block = ctx.enter_context(nc.Block())

        @block.sync
        def _(sync):
            # plain x0 load per instance; slot g%NMEM sees 4 loads/instance
            for g in range(ng):
                c = g % pb
                if g >= NMEM:
                    sync.wait_ge(v_sem, vidx_last[g - NMEM])
                sync.dma_start(mem[g % NMEM][:], xv[0, c]).then_inc(
                    mem_sem[g % NMEM], 16
                )

        @block.gpsimd
        def _(gp):
            # accumulate loads: mem[slot] += x_t, gated on mult(g, t-1)
            for g in range(ng):
                c = g % pb
                for t in range(1, t_dim):
                    gp.wait_ge(v_sem, vidx_mult[(g, t - 1)])
                    gp.dma_start(
                        mem[g % NMEM][:], xv[t, c], accum_op=mybir.AluOpType.add
                    ).then_inc(mem_sem[g % NMEM], 16)

        @block.vector
        def _(vector):
            v = 0

            def dve(ins):
                nonlocal v
                v += 1
                ins.then_inc(v_sem, 1)

            for g in range(ng):
                m = mem[g % NMEM]
                k = msk[g % NMEM]
                for t in range(t_dim):
                    u = t_dim * g + t
                    # mem slot has had 4*(g//NMEM) + t + 1 loads when step t ready
                    vector.wait_ge(
                        mem_sem[g % NMEM], 16 * (t_dim * (g // NMEM) + t + 1)
                    )
                    if u >= NSB:
                        vector.wait_ge(sb_sem[u % NSB], 16 * (u // NSB))
                    vector.wait_ge(v_sem, v)
                    dve(
                        vector.tensor_scalar(
                            sb[u % NSB][:], m[:], 1.0, None, mybir.AluOpType.is_ge
                        )
                    )
                    if t < t_dim - 1:
                        vector.wait_ge(v_sem, v)
                        dve(
                            vector.tensor_scalar(
                                k[:], m[:], 1.0, None, mybir.AluOpType.is_lt
                            )
                        )
                        vector.wait_ge(v_sem, v)
                        dve(vector.tensor_mul(m[:], m[:], k[:]))

        @block.scalar
        def _(scalar):
            for u in range(nu):
                g, t = divmod(u, t_dim)
                c = g % pb
                scalar.wait_ge(v_sem, vidx_ge[u])
                scalar.dma_start(ov[t, c], sb[u % NSB][:]).then_inc(
                    sb_sem[u % NSB], 16
                )
            for i in range(NSB):
                scalar.wait_ge(sb_sem[i], 16 * ((nu - 1 - i) // NSB + 1))

    return nc


_NC_CACHE = None


def _get_nc():
    global _NC_CACHE
    if _NC_CACHE is None:
        _NC_CACHE = build_nc()
    return _NC_CACHE


def shard_input(x):
    """x [T*B, N] -> list of 8 shards [T*BSH, N], C-contiguous."""
    xs = x.reshape(T, B, N)
    return [
        np.ascontiguousarray(xs[:, i * BSH : (i + 1) * BSH, :]).reshape(T * BSH, N)
        for i in range(N_CORES)
    ]


def unshard_output(results):
    """8 shards [T*BSH, N] -> full [T*B, N]."""
    out = np.empty((T, B, N), dtype=np.float32)
    for i in range(N_CORES):
        out[:, i * BSH : (i + 1) * BSH, :] = results[i].reshape(T, BSH, N)
    return out.reshape(T * B, N)


def run_sharded(x, trace=False):
    nc = _get_nc()
    in_maps = [{"x": s} for s in shard_input(x)]
    res = run_bass_kernel_spmd(nc, in_maps, list(range(N_CORES)), trace=trace)
    return unshard_output([r["out"] for r in res.results]), res


def kernel(x):
    x = np.asarray(x, dtype=np.float32)
    assert x.shape == (T * B, N)
    out, _ = run_sharded(x, trace=False)
    return out



# revision 4
# speedup vs baseline: 14.7363x; 14.7363x over previous
"""LIF spiking-neuron kernel v3 for Trainium2, data-parallel over 8 cores.

Reference semantics (T=4, THRESH=1.0, TAU=1.0):
    x: [T*B, N] -> reshape [T, B, N]; mem0 = 0
    per t: mem += x_t; spike_t = (mem >= 1.0); mem *= (1 - spike_t)
    out: spikes reshaped [T*B, N]

v3 = v2c + the ACT spike path. The spike (mem >= 1.0) is computed on
the otherwise-idle Activation engine as relu(sign(mem - c)) with
c = 1 - 2^-24 (largest f32 < 1). This is EXACT for every f32 mem:
HW-verified Sign(0) = 0, Sign(-0) = 0; there is no f32 strictly
between c and 1, so fl(mem - c) > 0 iff mem >= 1 and fl(mem - c) == 0
iff mem == c (no spike, correct); the affine inside ACT is a fused
mul-add (single rounding), and Sterbenz covers [0.5, 2] while sign
is trivially preserved outside.

Engine split per rep (per-core, [128, 4096] f32 ops, measured costs):
  DVE : 6 adds (4.33us) + 6 copy_predicated resets (4.4us) + 1 is_ge
        (1.9us)                                          ~= 54us
  ACT : 7 spikes x (Sign 3.7us + Relu->u8 3.7us)         ~= 52us
  GPSIMD (SWDGE queue): x loads (440 GB/s measured)      ~= 38us
  SYNC (HWDGE): u8 spike stores                          ~= 12us
DVE order [cpred_s(t), add_s(t+1)] alternating chunks makes the
DVE<->ACT ping-pong stall-free in steady state.
"""

from contextlib import ExitStack

import numpy as np

import concourse.bass as bass
from concourse import mybir
from concourse.bass_utils import run_bass_kernel_spmd

T = 4
B = 2048
N = 4096
N_CORES = 8
BSH = B // N_CORES  # 256 rows per core per timestep
P = 128

F32 = mybir.dt.float32
U8 = mybir.dt.uint8

# largest f32 strictly below 1.0
C_THRESH = float(np.float32(1.0) - np.float32(2.0**-24))


def build_nc(t_dim=T, bsh=BSH, n=N, bench_iters=None):
    """One-core Bass module: x [t*bsh, n] f32 -> out [t*bsh, n] u8."""
    pb = bsh // P
    assert bsh % P == 0 and pb == 2, "schedule written for pb=2"
    reps = bench_iters or 1
    NXB = 3  # x-tile ring (load-order indexed)
    NSP = 4  # u8 spike-tile ring (store-order indexed)
    NMEM = 2

    nc = bass.Bass()
    x = nc.declare_dram_parameter("x", [t_dim * bsh, n], F32, isOutput=False)
    out = nc.declare_dram_parameter("out", [t_dim * bsh, n], U8, isOutput=True)
    xv = x.rearrange("(t pb p) n -> t pb p n", t=t_dim, pb=pb, p=P)
    ov = out.rearrange("(t pb p) n -> t pb p n", t=t_dim, pb=pb, p=P)

    # Which (ci, t) spikes run on DVE instead of ACT: chunk B at t=T-1
    # (no downstream cpred, so it's a clean tail op).
    def on_dve(ci, t):
        return ci == 1 and t == t_dim - 1

    # ---- emission-order bookkeeping.
    # DVE: memset zz (v=1), memset cbias (v=2); per rep:
    #   for tau in 0..T-2: for ci: cpred(g,tau), add(g,tau+1)
    #   then ge(B, T-1)
    # ACT: per rep: t-major, ci inner: sign, relu  (skip the DVE one)
    # loads (gpsimd): t-major, ci inner
    # stores (sync): t-major, ci inner
    vidx_add = {}  # (g, t) -> v after add
    vidx_cpred = {}  # (g, t) -> v after cpred
    vidx_dge = {}  # g (odd) -> v after its t3 ge
    aidx_relu = {}  # (g, t) -> a count after relu
    su_of = {}  # (g, t) -> store order index
    k_of = {}  # (g, t) -> x-load order index (t >= 1)
    v = 2
    a = 2  # a=1,2: ACT warmup sign+relu (forces the table load early)
    su = 0
    k = 0
    for r in range(reps):
        for t in range(t_dim):
            for ci in range(pb):
                g = pb * r + ci
                su_of[(g, t)] = su
                su += 1
                if t > 0:
                    k_of[(g, t)] = k
                    k += 1
        for tau in range(t_dim - 1):
            for ci in range(pb):
                g = pb * r + ci
                v += 1
                vidx_cpred[(g, tau)] = v
                v += 1
                vidx_add[(g, tau + 1)] = v
        g_b = pb * r + 1
        v += 1
        vidx_dge[g_b] = v
        for t in range(t_dim):
            for ci in range(pb):
                if on_dve(ci, t):
                    continue
                a += 2  # sign, relu
                aidx_relu[(pb * r + ci, t)] = a

    with ExitStack() as ctx:
        mem = [
            ctx.enter_context(nc.sbuf_tensor(f"mem{i}", [P, n], F32))
            for i in range(NMEM)
        ]
        xb = [
            ctx.enter_context(nc.sbuf_tensor(f"xb{i}", [P, n], F32))
            for i in range(NXB)
        ]
        sp = [
            ctx.enter_context(nc.sbuf_tensor(f"sp{i}", [P, n], U8))
            for i in range(NSP)
        ]
        s1 = [
            ctx.enter_context(nc.sbuf_tensor(f"s1_{i}", [P, n], F32))
            for i in range(2)
        ]
        zz = ctx.enter_context(nc.sbuf_tensor("zz", [P, n], F32))
        cb = ctx.enter_context(nc.sbuf_tensor("cb", [P, 1], F32))
        mem_sem = [
            ctx.enter_context(nc.semaphore(f"mem_sem{i}")) for i in range(NMEM)
        ]
        xb_sem = [
            ctx.enter_context(nc.semaphore(f"xb_sem{i}")) for i in range(NXB)
        ]
        sp_sem = [
            ctx.enter_context(nc.semaphore(f"sp_sem{i}")) for i in range(NSP)
        ]
        v_sem = ctx.enter_context(nc.semaphore("v_sem"))
        a_sem = ctx.enter_context(nc.semaphore("a_sem"))
        block = ctx.enter_context(nc.Block())

        k_to_gt = {kk: gt for gt, kk in k_of.items()}

        @block.gpsimd
        def _(gp):
            for r in range(reps):
                for t in range(t_dim):
                    for ci in range(pb):
                        g = pb * r + ci
                        if t == 0:
                            if g >= NMEM:  # WAR: t3 spike of g-NMEM read mem
                                gp_prev = g - NMEM
                                if on_dve(ci, t_dim - 1):
                                    gp.wait_ge(v_sem, vidx_dge[gp_prev])
                                else:
                                    gp.wait_ge(
                                        a_sem, aidx_relu[(gp_prev, t_dim - 1)]
                                    )
                            gp.dma_start(mem[g % NMEM][:], xv[0, ci]).then_inc(
                                mem_sem[g % NMEM], 16
                            )
                        else:
                            kk = k_of[(g, t)]
                            if kk >= NXB:  # WAR: add of load kk-NXB done
                                gp.wait_ge(v_sem, vidx_add[k_to_gt[kk - NXB]])
                            gp.dma_start(xb[kk % NXB][:], xv[t, ci]).then_inc(
                                xb_sem[kk % NXB], 16
                            )

        @block.vector
        def _(vector):
            v = 0

            def dve(ins):
                nonlocal v
                v += 1
                ins.then_inc(v_sem, 1)

            dve(vector.memset(zz[:], 0.0))
            dve(vector.memset(cb[:], -C_THRESH))
            for r in range(reps):
                for tau in range(t_dim - 1):
                    for ci in range(pb):
                        g = pb * r + ci
                        m = mem[g % NMEM]
                        u = su_of[(g, tau)]
                        # cpred(g, tau): wait ACT's relu (spike tile ready)
                        vector.wait_ge(a_sem, aidx_relu[(g, tau)])
                        vector.wait_ge(v_sem, v)
                        dve(vector.copy_predicated(m[:], sp[u % NSP][:], zz[:]))
                        # add(g, tau+1)
                        kk = k_of[(g, tau + 1)]
                        vector.wait_ge(xb_sem[kk % NXB], 16 * (kk // NXB + 1))
                        vector.wait_ge(v_sem, v)
                        dve(vector.tensor_add(m[:], m[:], xb[kk % NXB][:]))
                # DVE-handled spike: chunk B, t = T-1
                g_b = pb * r + 1
                u = su_of[(g_b, t_dim - 1)]
                if u >= NSP:
                    vector.wait_ge(sp_sem[u % NSP], 16 * (u // NSP))
                vector.wait_ge(v_sem, v)
                dve(
                    vector.tensor_scalar(
                        sp[u % NSP][:],
                        mem[g_b % NMEM][:],
                        1.0,
                        None,
                        mybir.AluOpType.is_ge,
                    )
                )

        @block.scalar
        def _(scalar):
            a = 0

            def act(ins):
                nonlocal a
                a += 1
                ins.then_inc(a_sem, 1)

            # Warmup on the pre-initialized const-0 tile: triggers the ACT
            # function-table load before heavy DMA/sem traffic is in flight.
            c0 = nc.const_aps.tensor(0.0, (P, 1))
            act(scalar.activation(s1[0][:, 0:1], c0, mybir.ActivationFunctionType.Sign))
            scalar.wait_ge(a_sem, 1)
            act(scalar.activation(s1[1][:, 0:1], s1[0][:, 0:1], mybir.ActivationFunctionType.Relu))
            for r in range(reps):
                for t in range(t_dim):
                    for ci in range(pb):
                        if on_dve(ci, t):
                            continue
                        g = pb * r + ci
                        m = mem[g % NMEM]
                        u = su_of[(g, t)]
                        if t == 0:
                            scalar.wait_ge(mem_sem[g % NMEM], 16 * (g // NMEM + 1))
                            scalar.wait_ge(v_sem, 2)  # cb initialized
                        else:
                            scalar.wait_ge(v_sem, vidx_add[(g, t)])
                        if u >= NSP:  # WAR: store of u-NSP done
                            scalar.wait_ge(sp_sem[u % NSP], 16 * (u // NSP))
                        scalar.wait_ge(a_sem, a)
                        act(
                            scalar.activation(
                                s1[ci][:],
                                m[:],
                                mybir.ActivationFunctionType.Sign,
                                bias=cb[:, :],
                            )
                        )
                        scalar.wait_ge(a_sem, a)
                        act(
                            scalar.activation(
                                sp[u % NSP][:],
                                s1[ci][:],
                                mybir.ActivationFunctionType.Relu,
                            )
                        )

        @block.sync
        def _(sync):
            nu = 0
            for r in range(reps):
                for t in range(t_dim):
                    for ci in range(pb):
                        g = pb * r + ci
                        u = su_of[(g, t)]
                        if on_dve(ci, t):
                            sync.wait_ge(v_sem, vidx_dge[g])
                        else:
                            sync.wait_ge(a_sem, aidx_relu[(g, t)])
                        sync.dma_start(ov[t, ci], sp[u % NSP][:]).then_inc(
                            sp_sem[u % NSP], 16
                        )
                        nu += 1
            for i in range(NSP):  # drain
                sync.wait_ge(sp_sem[i], 16 * ((nu - 1 - i) // NSP + 1))

    return nc


_NC_CACHE = None


def _get_nc():
    global _NC_CACHE
    if _NC_CACHE is None:
        _NC_CACHE = build_nc()
    return _NC_CACHE


def shard_input(x):
    """x [T*B, N] -> list of 8 shards [T*BSH, N], C-contiguous."""
    xs = x.reshape(T, B, N)
    return [
        np.ascontiguousarray(xs[:, i * BSH : (i + 1) * BSH, :]).reshape(T * BSH, N)
        for i in range(N_CORES)
    ]


def unshard_output(results):
    """8 u8 shards [T*BSH, N] -> full f32 [T*B, N]."""
    out = np.empty((T, B, N), dtype=np.float32)
    for i in range(N_CORES):
        out[:, i * BSH : (i + 1) * BSH, :] = results[i].reshape(T, BSH, N)
    return out.reshape(T * B, N)


def run_sharded(x, trace=False):
    nc = _get_nc()
    in_maps = [{"x": s} for s in shard_input(x)]
    res = run_bass_kernel_spmd(nc, in_maps, list(range(N_CORES)), trace=trace)
    return unshard_output([r["out"] for r in res.results]), res


def kernel(x):
    x = np.asarray(x, dtype=np.float32)
    assert x.shape == (T * B, N)
    out, _ = run_sharded(x, trace=False)
    return out
